# revision 3
# baseline (speedup 1.0000x reference)
"""GCN block v2: SBUF-resident bf16 y replica, SBUF-source pair gather,
feature-major one-hot aggregation with dst-dinv folded into the selector,
epilogue fused into the PSUM-drain activation.

Math per layer (PyG GCNConv):
    y = dinv * (x @ W)            [dinv includes self-loop degree]
    aggT[f, m] = sum_{(u,v)} dinv[v] * y[u, f] + dinv[m] * y[m, f]
    x' = relu(aggT + b)           [drain activation, bias per partition]
Self-loops are handled natively: each tile's PSUM group opens with one
matmul of the slot-major y_own tile against (identity * dinv), so they
never enter the gather/transpose/selector pipeline.

Node storage layout (per core, "pair-major"): local slot s in [0, 12544)
maps to partition j=(s//2)%128, rank k=(s//2)//128, parity p=s%2. Tiles
(k, p) of 128 slots are the matmul/aggregation unit; y_sb holds the full
8-core replica as bf16 256B pair-elements: element (core c, rank k) at
partition j covers slots 2*(k*128+j)+{0,1} of core c, free range
[(c*49+k)*256B, +256B) = feats of even node then odd node.

Edge phase: edges bucketed by (dst core, dst tile g, src window w) where
w = src_core//4 selects one of two 25088-pair int16 gather windows.
dma_gather (SBUF source, transpose) returns msgT[q, slot]; per 128-slot
slice: PE transpose (identity matmul) -> ACT copy to bf16 -> two parity
selectors sel_par[slot, m] = (iota==dst_m_par)*dinv_dst on DVE -> two PE
matmuls accumulate aggT[f, m] in PSUM across the tile's slices of both
windows -> one ACT Relu+bias drain straight into next layer's xT.
"""

import numpy as np

N = 100000
F = 64
NC = 8
NLOC = 12500          # real nodes per core
NP = 12544            # padded slots per core (49 ranks * 256)
KR = 49               # ranks per core
NT = 98               # tiles per core = KR * 2 parities
NWIN = 2              # gather windows (25088 pairs each, int16 limit)
WINP = 25088          # pairs per window
KMAX = 6144           # max gather slots per instruction
DEPTH = 4
ABLATE = set()


def _wrap16(idx, k, pad):
    padded = np.full(k, pad, np.int16)
    padded[: len(idx)] = idx
    blk = padded.reshape(k // 16, 16).T
    return np.tile(blk, (8, 1))


def _build_schedule(edge_index, dinv_full):
    """Bucket edges (incl. self-loops) by (dst core, dst tile, src window).

    Returns (pairs, blobs) where pairs is a list of
    (k0, k1, tiles=[(g, nsl0, nsl1), ...]) shared by all cores and
    blobs[c][i] = dict(gA, gB, qe, qo, dd) per core c, pair-chunk i.
    """
    src = np.asarray(edge_index[0], np.int64)
    dst = np.asarray(edge_index[1], np.int64)

    c = dst // NLOC
    sv = dst - c * NLOC
    g_d = (sv // 2) // 128 * 2 + (sv % 2)          # dst tile in [0, 98)
    m_d = (sv // 2) % 128                          # one-hot row
    cs = src // NLOC
    su = src - cs * NLOC
    w = cs // 4                                    # window
    idx16 = ((cs - 4 * w) * KR + (su // 2) // 128) * 128 + (su // 2) % 128
    par = su % 2
    dd = dinv_full[dst].astype(np.float32)

    key = ((c * NT + g_d) * NWIN + w)
    order = np.argsort(key, kind="stable")
    ks = key[order]
    uniq, start = np.unique(ks, return_index=True)
    start = np.append(start, len(ks))
    buckets = {}
    cnt = np.zeros((NC, NT, NWIN), np.int64)
    for i, kk in enumerate(uniq):
        cc, r = divmod(int(kk), NT * NWIN)
        g, ww = divmod(r, NWIN)
        a, b = start[i], start[i + 1]
        sel = order[a:b]
        buckets[(cc, g, ww)] = (idx16[sel], m_d[sel], par[sel], dd[sel])
        cnt[cc, g, ww] = b - a

    gmax = ((cnt.max(axis=0) + 127) // 128) * 128   # [NT, NWIN] slots

    # pack tiles into chunk-pairs (w0 chunk + w1 chunk, each <= KMAX)
    pairs = []
    cur, c0, c1 = [], 0, 0
    for g in range(NT):
        s0, s1 = int(gmax[g, 0]), int(gmax[g, 1])
        if c0 + s0 > KMAX or c1 + s1 > KMAX:
            pairs.append((c0, c1, cur))
            cur, c0, c1 = [], 0, 0
        cur.append((g, s0 // 128, s1 // 128))
        c0 += s0
        c1 += s1
    if cur:
        pairs.append((c0, c1, cur))

    import ml_dtypes
    blobs = [[] for _ in range(NC)]
    for cc in range(NC):
        for (k0, k1, tiles) in pairs:
            nsl_tot = (k0 + k1) // 128
            gsl = [np.zeros(k0, np.int16), np.zeros(k1, np.int16)]
            qe = np.full((128, nsl_tot), -1.0, np.float32)
            qo = np.full((128, nsl_tot), -1.0, np.float32)
            ddb = np.zeros((128, nsl_tot), np.float32)
            pos = [0, 0]
            col = 0
            for g, n0, n1 in tiles:
                for ww, nsl in ((0, n0), (1, n1)):
                    bkt = buckets.get((cc, g, ww))
                    nreal = 0 if bkt is None else len(bkt[0])
                    if nsl == 0:
                        continue
                    p0 = pos[ww]
                    if nreal:
                        gsl[ww][p0:p0 + nreal] = bkt[0]
                        mm, pp, dv = bkt[1], bkt[2], bkt[3]
                        sl = np.arange(nreal)
                        qe[sl % 128, col + sl // 128] = np.where(
                            pp == 0, mm, -1).astype(np.float32)
                        qo[sl % 128, col + sl // 128] = np.where(
                            pp == 1, mm, -1).astype(np.float32)
                        ddb[sl % 128, col + sl // 128] = dv
                    pos[ww] += nsl * 128
                    col += nsl
            b = {"gA": _wrap16(gsl[0], max(k0, 16), 0),
                 "gB": _wrap16(gsl[1], max(k1, 16), 0),
                 "qe": qe, "qo": qo, "dd": ddb}
            blobs[cc].append(b)
    return pairs, blobs


def _build_program(pairs):
    from concourse import bacc, tile
    from concourse import mybir

    f32, i16, bf16 = mybir.dt.float32, mybir.dt.int16, mybir.dt.bfloat16
    nc = bacc.Bacc("TRN2", target_bir_lowering=False, debug=False,
                   num_devices=NC, num_swdge_queues=2)

    NPAIR = len(pairs)
    xt_in = nc.dram_tensor("xt", [64, NP], bf16, kind="ExternalInput")
    w_in = nc.dram_tensor("W", [F, F], f32, kind="ExternalInput")
    b_in = nc.dram_tensor("b", [64, 1], f32, kind="ExternalInput")
    id_in = nc.dram_tensor("ident", [128, 128], f32, kind="ExternalInput")
    iota_in = nc.dram_tensor("iota", [128, 128], f32, kind="ExternalInput")
    dinv_in = nc.dram_tensor("dinvc", [128, NT], f32, kind="ExternalInput")
    gA_in = [nc.dram_tensor(f"gA{i}", [128, max(k0, 16) // 16], i16,
                            kind="ExternalInput")
             for i, (k0, k1, _) in enumerate(pairs)]
    gB_in = [nc.dram_tensor(f"gB{i}", [128, max(k1, 16) // 16], i16,
                            kind="ExternalInput")
             for i, (k0, k1, _) in enumerate(pairs)]
    qe_in = [nc.dram_tensor(f"qe{i}", [128, (k0 + k1) // 128], f32,
                            kind="ExternalInput")
             for i, (k0, k1, _) in enumerate(pairs)]
    qo_in = [nc.dram_tensor(f"qo{i}", [128, (k0 + k1) // 128], f32,
                            kind="ExternalInput")
             for i, (k0, k1, _) in enumerate(pairs)]
    dd_in = [nc.dram_tensor(f"dd{i}", [128, (k0 + k1) // 128], f32,
                            kind="ExternalInput")
             for i, (k0, k1, _) in enumerate(pairs)]
    out_d = nc.dram_tensor("out", [128, NT * F], f32, kind="ExternalOutput")

    y_loc = nc.dram_tensor("y_loc", [128, NP // 2], bf16)
    y_full = [nc.dram_tensor(f"y_full{i}", [NC * 128, NP // 2], bf16,
                             addr_space="Shared") for i in range(2)]

    Copy = mybir.ActivationFunctionType.Copy
    Relu = mybir.ActivationFunctionType.Relu
    iseq = mybir.AluOpType.is_equal
    mult = mybir.AluOpType.mult

    with tile.TileContext(nc) as tc:
        with tc.tile_pool(name="persist", bufs=1) as pp, \
             tc.tile_pool(name="msg", bufs=2) as mp, \
             tc.tile_pool(name="sb", bufs=8) as sp_, \
             tc.tile_pool(name="idx", bufs=3) as ip, \
             tc.tile_pool(name="pst", bufs=3, space="PSUM") as qt, \
             tc.tile_pool(name="psa", bufs=4, space="PSUM") as qa, \
             tc.tile_pool(name="psm", bufs=1, space="PSUM") as qm:

            y_sb = pp.tile([128, NC * NP // 2], bf16)     # full replica
            y_own = pp.tile([128, NP // 2], bf16)
            xT = pp.tile([64, NP], bf16)
            w_sb = pp.tile([F, F], bf16)
            w_f32 = pp.tile([F, F], f32)
            b_sb = pp.tile([64, 1], f32)
            id_sb = pp.tile([128, 128], bf16)
            id_f32 = pp.tile([128, 128], f32)
            iota_sb = pp.tile([128, 128], bf16)
            iota_f32 = pp.tile([128, 128], f32)
            dinvc = pp.tile([128, NT], f32)

            nc.sync.dma_start(w_f32[:], w_in[:])
            nc.vector.tensor_copy(w_sb[:], w_f32[:])
            nc.sync.dma_start(b_sb[:], b_in[:])
            nc.sync.dma_start(id_f32[:], id_in[:])
            nc.vector.tensor_copy(id_sb[:], id_f32[:])
            nc.sync.dma_start(iota_f32[:], iota_in[:])
            nc.vector.tensor_copy(iota_sb[:], iota_f32[:])
            nc.sync.dma_start(dinvc[:], dinv_in[:])
            nc.sync.dma_start(xT[:], xt_in[:])

            for l in range(DEPTH):
                yf = y_full[l % 2]
                # y = dinv * (x @ W), bf16, pair-element layout
                with tc.nc.named_scope(f"L{l}_y"):
                    for g in range(NT):
                        h = qm.tile([128, F], f32, tag="h")
                        nc.tensor.matmul(h[:], xT[:, g * 128:(g + 1) * 128],
                                         w_sb[:], start=True, stop=True)
                        nc.scalar.activation(
                            y_own[:, g * 64:(g + 1) * 64], h[:], Copy,
                            scale=dinvc[:, g:g + 1])
                    nc.scalar.dma_start(y_loc[:], y_own[:])
                with tc.nc.named_scope(f"L{l}_ag"):
                    if 'cc' not in ABLATE:
                        nc.gpsimd.collective_compute(
                            "AllGather", mybir.AluOpType.bypass,
                            replica_groups=[list(range(NC))],
                            ins=[y_loc[:]], outs=[yf[:]])
                with tc.nc.named_scope(f"L{l}_load"):
                    for cb in range(NC):
                        nc.sync.dma_start(
                            y_sb[:, cb * (NP // 2):(cb + 1) * (NP // 2)],
                            yf[cb * 128:(cb + 1) * 128, :])
                    # SWDGE gathers read y_sb without a tracked dep; stall the
                    # in-order Pool sequencer on each block load's completion so
                    # no gather's descriptor generation can start before the
                    # loads land
                    ysc = sp_.tile([128, NC], bf16, tag="ysc")
                    for cb in range(NC):
                        nc.gpsimd.tensor_copy(
                            ysc[:, cb:cb + 1],
                            y_sb[:, cb * (NP // 2):cb * (NP // 2) + 1])

                # edge phase
                for pi in range(NPAIR if 'edges' not in ABLATE else 0):
                  with tc.nc.named_scope(f"L{l}_edge"):
                    k0, k1, tiles = pairs[pi]
                    nsl_tot = (k0 + k1) // 128
                    gAt = ip.tile([128, KMAX // 16], i16, tag="gA")
                    gBt = ip.tile([128, KMAX // 16], i16, tag="gB")
                    qet = ip.tile([128, KMAX // 64], f32, tag="qe")
                    qot = ip.tile([128, KMAX // 64], f32, tag="qo")
                    ddt = ip.tile([128, KMAX // 64], f32, tag="dd")
                    nc.sync.dma_start(gAt[:, : max(k0, 16) // 16], gA_in[pi][:])
                    nc.sync.dma_start(gBt[:, : max(k1, 16) // 16], gB_in[pi][:])
                    nc.sync.dma_start(qet[:, : nsl_tot], qe_in[pi][:])
                    nc.sync.dma_start(qot[:, : nsl_tot], qo_in[pi][:])
                    nc.sync.dma_start(ddt[:, : nsl_tot], dd_in[pi][:])
                    mt = []
                    for ww, kk, gt in ((0, k0, gAt), (1, k1, gBt)):
                        if kk == 0:
                            mt.append(None)
                            continue
                        m = mp.tile([128, 1, KMAX], bf16, tag=f"m{ww}")
                        nc.gpsimd.dma_gather(
                            m[:, :, :kk],
                            y_sb[:, ww * WINP:(ww + 1) * WINP],
                            gt[:, : kk // 16], kk, kk, 128,
                            transpose=True, single_packet=False,
                            queue_num=ww,
                            sbuf_tokens_per_rank=128,
                            sbuf_free_dim_per_rank=256)
                        mt.append(m)
                    col = 0
                    off = [0, 0]
                    for g, n0, n1 in tiles:
                        agg = qa.tile([64, 128], f32, tag="agg")
                        # self-loop: msg rows are already slot-major in
                        # y_own; selector = identity * dinv_dst
                        seld = sp_.tile([128, 128], bf16, tag="seld")
                        nc.vector.tensor_scalar(
                            seld[:], id_sb[:], dinvc[:, g:g + 1],
                            None, mult)
                        nsl = n0 + n1
                        nc.tensor.matmul(agg[:],
                                         y_own[:, g * 64:(g + 1) * 64],
                                         seld[:], start=True,
                                         stop=(nsl == 0))
                        u = 0
                        for ww, nn in ((0, n0), (1, n1)):
                            for _ in range(nn):
                                j = off[ww]
                                msgT = mt[ww][:, 0, j * 128:(j + 1) * 128]
                                tr = qt.tile([128, 128], f32, tag="tr")
                                nc.tensor.matmul(tr[:], msgT, id_sb[:],
                                                 start=True, stop=True)
                                msb = sp_.tile([128, 128], bf16, tag="msb")
                                nc.scalar.activation(msb[:], tr[:], Copy)
                                sele = sp_.tile([128, 128], bf16, tag="sele")
                                nc.vector.tensor_scalar(
                                    sele[:], iota_sb[:], qet[:, col:col + 1],
                                    ddt[:, col:col + 1], iseq, mult)
                                selo = sp_.tile([128, 128], bf16, tag="selo")
                                nc.gpsimd.tensor_scalar(
                                    selo[:], iota_sb[:], qot[:, col:col + 1],
                                    ddt[:, col:col + 1], iseq, mult)
                                nc.tensor.matmul(agg[:], msb[:, 0:64],
                                                 sele[:],
                                                 start=False, stop=False)
                                nc.tensor.matmul(agg[:], msb[:, 64:128],
                                                 selo[:],
                                                 start=False,
                                                 stop=(u == nsl - 1))
                                col += 1
                                off[ww] += 1
                                u += 1
                        # drain: x' = relu(agg + b)
                        if l < DEPTH - 1:
                            nc.scalar.activation(
                                xT[:, g * 128:(g + 1) * 128], agg[:],
                                Relu, bias=b_sb[:])
                        else:
                            fr = sp_.tile([64, 128], f32, tag="fr")
                            nc.scalar.activation(fr[:], agg[:], Relu,
                                                 bias=b_sb[:])
                            trf = qt.tile([128, 64], f32, tag="tr")
                            nc.tensor.matmul(trf[:], fr[:],
                                             id_f32[0:64, 0:64],
                                             start=True, stop=True)
                            ot = sp_.tile([128, 64], f32, tag="ot")
                            nc.vector.tensor_copy(ot[:], trf[:])
                            nc.scalar.dma_start(
                                out_d[:, g * F:(g + 1) * F], ot[:])

    nc.compile()
    return nc


def _host_inputs(x, W, b, edge_index):
    import ml_dtypes
    deg_full = np.bincount(
        np.asarray(edge_index[1], np.int64), minlength=N
    ).astype(np.float64) + 1.0
    dinv_full = (1.0 / np.sqrt(deg_full)).astype(np.float32)
    pairs, blobs = _build_schedule(np.asarray(edge_index), dinv_full)

    ident = np.eye(128, dtype=np.float32)
    iota = np.tile(np.arange(128, dtype=np.float32)[None, :], (128, 1))
    x = np.asarray(x, np.float32)

    # slot s (pair-major) <-> xT column (2k+p)*128 + j
    s_of_col = np.empty(NP, np.int64)
    for g in range(NT):
        k, p = g // 2, g % 2
        j = np.arange(128)
        s_of_col[g * 128 + j] = 256 * k + 2 * j + p

    in_maps = []
    for c in range(NC):
        xp = np.zeros((NP, F), np.float32)
        xp[:NLOC] = x[c * NLOC:(c + 1) * NLOC]
        xt = xp[s_of_col].T                      # [64, NP]
        dg = np.ones(NP, np.float32)
        dg_loc = dinv_full[c * NLOC:(c + 1) * NLOC]
        dg[:NLOC] = dg_loc
        dinv_slot = dg                           # indexed by slot
        dinvc = np.empty((128, NT), np.float32)
        for g in range(NT):
            dinvc[:, g] = dinv_slot[s_of_col[g * 128:(g + 1) * 128]]
        m = {"xt": np.ascontiguousarray(xt).astype(ml_dtypes.bfloat16),
             "W": np.asarray(W, np.float32),
             "b": np.asarray(b, np.float32).reshape(64, 1),
             "ident": ident, "iota": iota,
             "dinvc": np.ascontiguousarray(dinvc)}
        for i, bl in enumerate(blobs[c]):
            m[f"gA{i}"] = bl["gA"]
            m[f"gB{i}"] = bl["gB"]
            m[f"qe{i}"] = bl["qe"]
            m[f"qo{i}"] = bl["qo"]
            m[f"dd{i}"] = bl["dd"]
        in_maps.append(m)
    return pairs, in_maps, s_of_col


def unpack(out_core):
    """[128, NT*F] device layout -> [NLOC, F] node-major."""
    o = out_core.reshape(128, NT, F)
    res = np.empty((NP, F), np.float32)
    j = np.arange(128)
    for g in range(NT):
        k, p = g // 2, g % 2
        res[256 * k + 2 * j + p] = o[j, g]
    return res[:NLOC]


last_results = None
last_exec_ns = None


def kernel(x, edge_index, batch_index, node_rankings, W, b, **_unused):
    import os
    import time
    from concourse.bass_utils import run_bass_kernel_spmd

    global last_results, last_exec_ns
    pairs, in_maps, _ = _host_inputs(x, W, b, np.asarray(edge_index))
    nc = _build_program(pairs)

    cores = list(range(NC))
    captured = {}
    if os.environ.get("KERNEL_TIME"):
        import jax
        orig_jit = jax.jit

        def spy_jit(*a, **kw):
            f = orig_jit(*a, **kw)

            def wrapper(*args):
                captured["fn"], captured["args"] = f, args
                return f(*args)
            return wrapper
        jax.jit = spy_jit
    try:
        if os.environ.get("KERNEL_TRACE"):
            try:
                res = run_bass_kernel_spmd(nc, in_maps, cores, trace=True)
            except Exception:
                res = run_bass_kernel_spmd(nc, in_maps, cores)
        else:
            res = run_bass_kernel_spmd(nc, in_maps, cores)
    finally:
        if captured:
            import jax
            jax.jit = orig_jit
    if captured.get("fn") is not None:
        # warm re-execution of the captured jitted NEFF call: wall time is
        # upload + execute + sync, no retrace/compile
        import jax
        t0 = time.perf_counter()
        o = captured["fn"](*captured["args"])
        jax.block_until_ready(o)
        last_exec_ns = int((time.perf_counter() - t0) * 1e9)
    last_results = res

    out = np.empty((N, F), np.float32)
    for c in range(NC):
        out[c * NLOC:(c + 1) * NLOC] = unpack(res.results[c]["out"])
    return out



# revision 4
# speedup vs baseline: 54.2262x; 54.2262x over previous
"""GCN block v2: SBUF-resident bf16 y replica, SBUF-source pair gather,
feature-major one-hot aggregation with dst-dinv folded into the selector,
epilogue fused into the PSUM-drain activation.

Math per layer (PyG GCNConv):
    y = dinv * (x @ W)            [dinv includes self-loop degree]
    aggT[f, m] = sum_{(u,v)} dinv[v] * y[u, f] + dinv[m] * y[m, f]
    x' = relu(aggT + b)           [drain activation, bias per partition]
Self-loops are handled natively: each tile's PSUM group opens with one
matmul of the slot-major y_own tile against (identity * dinv), so they
never enter the gather/transpose/selector pipeline.

Node storage layout (per core, "pair-major"): local slot s in [0, 12544)
maps to partition j=(s//2)%128, rank k=(s//2)//128, parity p=s%2. Tiles
(k, p) of 128 slots are the matmul/aggregation unit; y_sb holds the full
8-core replica as bf16 256B pair-elements: element (core c, rank k) at
partition j covers slots 2*(k*128+j)+{0,1} of core c, free range
[(c*49+k)*256B, +256B) = feats of even node then odd node.

Edge phase: edges bucketed by (dst core, dst tile g, src window w) where
w = src_core//4 selects one of two 25088-pair int16 gather windows.
dma_gather (SBUF source, transpose) returns msgT[q, slot]; per 128-slot
slice: PE transpose (identity matmul) -> ACT copy to bf16 -> two parity
selectors sel_par[slot, m] = (iota==dst_m_par)*dinv_dst on DVE -> two PE
matmuls accumulate aggT[f, m] in PSUM across the tile's slices of both
windows -> one ACT Relu+bias drain straight into next layer's xT.
"""

import numpy as np

N = 100000
F = 64
NC = 8
NLOC = 12500          # real nodes per core
NP = 12544            # padded slots per core (49 ranks * 256)
KR = 49               # ranks per core
NT = 98               # tiles per core = KR * 2 parities
NWIN = 2              # gather windows (25088 pairs each, int16 limit)
WINP = 25088          # pairs per window
KMAX = 6144           # max gather slots per instruction
DEPTH = 4
ABLATE = set()


def _wrap16(idx, k, pad):
    padded = np.full(k, pad, np.int16)
    padded[: len(idx)] = idx
    blk = padded.reshape(k // 16, 16).T
    return np.tile(blk, (8, 1))


def _build_schedule(edge_index, dinv_full):
    """Bucket edges (incl. self-loops) by (dst core, dst tile, src window).

    Returns (pairs, blobs) where pairs is a list of
    (k0, k1, tiles=[(g, nsl0, nsl1), ...]) shared by all cores and
    blobs[c][i] = dict(gA, gB, qe, qo, dd) per core c, pair-chunk i.
    """
    src = np.asarray(edge_index[0], np.int64)
    dst = np.asarray(edge_index[1], np.int64)

    c = dst // NLOC
    sv = dst - c * NLOC
    g_d = (sv // 2) // 128 * 2 + (sv % 2)          # dst tile in [0, 98)
    m_d = (sv // 2) % 128                          # one-hot row
    cs = src // NLOC
    su = src - cs * NLOC
    w = cs // 4                                    # window
    idx16 = ((cs - 4 * w) * KR + (su // 2) // 128) * 128 + (su // 2) % 128
    par = su % 2
    dd = dinv_full[dst].astype(np.float32)

    key = ((c * NT + g_d) * NWIN + w)
    order = np.argsort(key, kind="stable")
    ks = key[order]
    uniq, start = np.unique(ks, return_index=True)
    start = np.append(start, len(ks))
    buckets = {}
    cnt = np.zeros((NC, NT, NWIN), np.int64)
    for i, kk in enumerate(uniq):
        cc, r = divmod(int(kk), NT * NWIN)
        g, ww = divmod(r, NWIN)
        a, b = start[i], start[i + 1]
        sel = order[a:b]
        buckets[(cc, g, ww)] = (idx16[sel], m_d[sel], par[sel], dd[sel])
        cnt[cc, g, ww] = b - a

    gmax = ((cnt.max(axis=0) + 127) // 128) * 128   # [NT, NWIN] slots

    # pack tiles into chunk-pairs (w0 chunk + w1 chunk, each <= KMAX)
    pairs = []
    cur, c0, c1 = [], 0, 0
    for g in range(NT):
        s0, s1 = int(gmax[g, 0]), int(gmax[g, 1])
        if c0 + s0 > KMAX or c1 + s1 > KMAX:
            pairs.append((c0, c1, cur))
            cur, c0, c1 = [], 0, 0
        cur.append((g, s0 // 128, s1 // 128))
        c0 += s0
        c1 += s1
    if cur:
        pairs.append((c0, c1, cur))

    import ml_dtypes
    blobs = [[] for _ in range(NC)]
    for cc in range(NC):
        for (k0, k1, tiles) in pairs:
            nsl_tot = (k0 + k1) // 128
            gsl = [np.zeros(k0, np.int16), np.zeros(k1, np.int16)]
            qe = np.full((128, nsl_tot), -1.0, np.float32)
            qo = np.full((128, nsl_tot), -1.0, np.float32)
            ddb = np.zeros((128, nsl_tot), np.float32)
            pos = [0, 0]
            col = 0
            for g, n0, n1 in tiles:
                for ww, nsl in ((0, n0), (1, n1)):
                    bkt = buckets.get((cc, g, ww))
                    nreal = 0 if bkt is None else len(bkt[0])
                    if nsl == 0:
                        continue
                    p0 = pos[ww]
                    if nreal:
                        gsl[ww][p0:p0 + nreal] = bkt[0]
                        mm, pp, dv = bkt[1], bkt[2], bkt[3]
                        sl = np.arange(nreal)
                        qe[sl % 128, col + sl // 128] = np.where(
                            pp == 0, mm, -1).astype(np.float32)
                        qo[sl % 128, col + sl // 128] = np.where(
                            pp == 1, mm, -1).astype(np.float32)
                        ddb[sl % 128, col + sl // 128] = dv
                    pos[ww] += nsl * 128
                    col += nsl
            b = {"gA": _wrap16(gsl[0], max(k0, 16), 0),
                 "gB": _wrap16(gsl[1], max(k1, 16), 0),
                 "qe": qe, "qo": qo, "dd": ddb}
            blobs[cc].append(b)
    return pairs, blobs


def _build_program(pairs):
    from concourse import bacc, tile
    from concourse import mybir

    f32, i16, bf16 = mybir.dt.float32, mybir.dt.int16, mybir.dt.bfloat16
    nc = bacc.Bacc("TRN2", target_bir_lowering=False, debug=False,
                   num_devices=NC, num_swdge_queues=2)

    NPAIR = len(pairs)
    xt_in = nc.dram_tensor("xt", [64, NP], bf16, kind="ExternalInput")
    w_in = nc.dram_tensor("W", [F, F], f32, kind="ExternalInput")
    b_in = nc.dram_tensor("b", [64, 1], f32, kind="ExternalInput")
    id_in = nc.dram_tensor("ident", [128, 128], f32, kind="ExternalInput")
    iota_in = nc.dram_tensor("iota", [128, 128], f32, kind="ExternalInput")
    dinv_in = nc.dram_tensor("dinvc", [128, NT], f32, kind="ExternalInput")
    gA_in = [nc.dram_tensor(f"gA{i}", [128, max(k0, 16) // 16], i16,
                            kind="ExternalInput")
             for i, (k0, k1, _) in enumerate(pairs)]
    gB_in = [nc.dram_tensor(f"gB{i}", [128, max(k1, 16) // 16], i16,
                            kind="ExternalInput")
             for i, (k0, k1, _) in enumerate(pairs)]
    qe_in = [nc.dram_tensor(f"qe{i}", [128, (k0 + k1) // 128], f32,
                            kind="ExternalInput")
             for i, (k0, k1, _) in enumerate(pairs)]
    qo_in = [nc.dram_tensor(f"qo{i}", [128, (k0 + k1) // 128], f32,
                            kind="ExternalInput")
             for i, (k0, k1, _) in enumerate(pairs)]
    dd_in = [nc.dram_tensor(f"dd{i}", [128, (k0 + k1) // 128], f32,
                            kind="ExternalInput")
             for i, (k0, k1, _) in enumerate(pairs)]
    out_d = nc.dram_tensor("out", [128, NT * F], f32, kind="ExternalOutput")

    y_loc = nc.dram_tensor("y_loc", [128, NP // 2], bf16)
    y_full = [nc.dram_tensor(f"y_full{i}", [NC * 128, NP // 2], bf16,
                             addr_space="Shared") for i in range(2)]

    Copy = mybir.ActivationFunctionType.Copy
    Relu = mybir.ActivationFunctionType.Relu
    iseq = mybir.AluOpType.is_equal
    mult = mybir.AluOpType.mult

    with tile.TileContext(nc) as tc:
        with tc.tile_pool(name="persist", bufs=1) as pp, \
             tc.tile_pool(name="msg", bufs=2) as mp, \
             tc.tile_pool(name="sb", bufs=8) as sp_, \
             tc.tile_pool(name="idx", bufs=3) as ip, \
             tc.tile_pool(name="pst", bufs=3, space="PSUM") as qt, \
             tc.tile_pool(name="psa", bufs=4, space="PSUM") as qa, \
             tc.tile_pool(name="psm", bufs=1, space="PSUM") as qm:

            y_sb = pp.tile([128, NC * NP // 2], bf16)     # full replica
            y_own = pp.tile([128, NP // 2], bf16)
            xT = pp.tile([64, NP], bf16)
            w_sb = pp.tile([F, F], bf16)
            w_f32 = pp.tile([F, F], f32)
            b_sb = pp.tile([64, 1], f32)
            id_sb = pp.tile([128, 128], bf16)
            id_f32 = pp.tile([128, 128], f32)
            iota_sb = pp.tile([128, 128], bf16)
            iota_f32 = pp.tile([128, 128], f32)
            dinvc = pp.tile([128, NT], f32)

            nc.sync.dma_start(w_f32[:], w_in[:])
            nc.vector.tensor_copy(w_sb[:], w_f32[:])
            nc.sync.dma_start(b_sb[:], b_in[:])
            nc.sync.dma_start(id_f32[:], id_in[:])
            nc.vector.tensor_copy(id_sb[:], id_f32[:])
            nc.sync.dma_start(iota_f32[:], iota_in[:])
            nc.vector.tensor_copy(iota_sb[:], iota_f32[:])
            nc.sync.dma_start(dinvc[:], dinv_in[:])
            nc.sync.dma_start(xT[:], xt_in[:])

            for l in range(DEPTH):
                yf = y_full[l % 2]
                # y = dinv * (x @ W), bf16, pair-element layout
                with tc.nc.named_scope(f"L{l}_y"):
                    for g in range(NT):
                        h = qm.tile([128, F], f32, tag="h")
                        nc.tensor.matmul(h[:], xT[:, g * 128:(g + 1) * 128],
                                         w_sb[:], start=True, stop=True)
                        nc.scalar.activation(
                            y_own[:, g * 64:(g + 1) * 64], h[:], Copy,
                            scale=dinvc[:, g:g + 1])
                    nc.scalar.dma_start(y_loc[:], y_own[:])
                with tc.nc.named_scope(f"L{l}_ag"):
                    if 'cc' not in ABLATE:
                        nc.gpsimd.collective_compute(
                            "AllGather", mybir.AluOpType.bypass,
                            replica_groups=[list(range(NC))],
                            ins=[y_loc[:]], outs=[yf[:]])
                with tc.nc.named_scope(f"L{l}_load"):
                    for cb in range(NC):
                        nc.sync.dma_start(
                            y_sb[:, cb * (NP // 2):(cb + 1) * (NP // 2)],
                            yf[cb * 128:(cb + 1) * 128, :])
                    # SWDGE gathers read y_sb without a tracked dep; stall the
                    # in-order Pool sequencer on each block load's completion so
                    # no gather's descriptor generation can start before the
                    # loads land
                    ysc = sp_.tile([128, NC], bf16, tag="ysc")
                    for cb in range(NC):
                        nc.gpsimd.tensor_copy(
                            ysc[:, cb:cb + 1],
                            y_sb[:, cb * (NP // 2):cb * (NP // 2) + 1])

                # edge phase
                for pi in range(NPAIR if 'edges' not in ABLATE else 0):
                  with tc.nc.named_scope(f"L{l}_edge"):
                    k0, k1, tiles = pairs[pi]
                    nsl_tot = (k0 + k1) // 128
                    gAt = ip.tile([128, KMAX // 16], i16, tag="gA")
                    gBt = ip.tile([128, KMAX // 16], i16, tag="gB")
                    qet = ip.tile([128, KMAX // 64], f32, tag="qe")
                    qot = ip.tile([128, KMAX // 64], f32, tag="qo")
                    ddt = ip.tile([128, KMAX // 64], f32, tag="dd")
                    nc.sync.dma_start(gAt[:, : max(k0, 16) // 16], gA_in[pi][:])
                    nc.sync.dma_start(gBt[:, : max(k1, 16) // 16], gB_in[pi][:])
                    nc.sync.dma_start(qet[:, : nsl_tot], qe_in[pi][:])
                    nc.sync.dma_start(qot[:, : nsl_tot], qo_in[pi][:])
                    nc.sync.dma_start(ddt[:, : nsl_tot], dd_in[pi][:])
                    mt = []
                    for ww, kk, gt in ((0, k0, gAt), (1, k1, gBt)):
                        if kk == 0:
                            mt.append(None)
                            continue
                        m = mp.tile([128, 1, KMAX], bf16, tag=f"m{ww}")
                        nc.gpsimd.dma_gather(
                            m[:, :, :kk],
                            y_sb[:, ww * WINP:(ww + 1) * WINP],
                            gt[:, : kk // 16], kk, kk, 128,
                            transpose=True, single_packet=False,
                            queue_num=ww,
                            sbuf_tokens_per_rank=128,
                            sbuf_free_dim_per_rank=256)
                        mt.append(m)
                    col = 0
                    off = [0, 0]
                    for g, n0, n1 in tiles:
                        agg = qa.tile([64, 128], f32, tag="agg")
                        # self-loop: msg rows are already slot-major in
                        # y_own; selector = identity * dinv_dst
                        seld = sp_.tile([128, 128], bf16, tag="seld")
                        nc.vector.tensor_scalar(
                            seld[:], id_sb[:], dinvc[:, g:g + 1],
                            None, mult)
                        nsl = n0 + n1
                        nc.tensor.matmul(agg[:],
                                         y_own[:, g * 64:(g + 1) * 64],
                                         seld[:], start=True,
                                         stop=(nsl == 0))
                        u = 0
                        for ww, nn in ((0, n0), (1, n1)):
                            for _ in range(nn):
                                j = off[ww]
                                msgT = mt[ww][:, 0, j * 128:(j + 1) * 128]
                                tr = qt.tile([128, 128], f32, tag="tr")
                                nc.tensor.matmul(tr[:], msgT, id_sb[:],
                                                 start=True, stop=True)
                                msb = sp_.tile([128, 128], bf16, tag="msb")
                                nc.scalar.activation(msb[:], tr[:], Copy)
                                sele = sp_.tile([128, 128], bf16, tag="sele")
                                nc.vector.tensor_scalar(
                                    sele[:], iota_sb[:], qet[:, col:col + 1],
                                    ddt[:, col:col + 1], iseq, mult)
                                selo = sp_.tile([128, 128], bf16, tag="selo")
                                nc.gpsimd.tensor_scalar(
                                    selo[:], iota_sb[:], qot[:, col:col + 1],
                                    ddt[:, col:col + 1], iseq, mult)
                                nc.tensor.matmul(agg[:], msb[:, 0:64],
                                                 sele[:],
                                                 start=False, stop=False)
                                nc.tensor.matmul(agg[:], msb[:, 64:128],
                                                 selo[:],
                                                 start=False,
                                                 stop=(u == nsl - 1))
                                col += 1
                                off[ww] += 1
                                u += 1
                        # drain: x' = relu(agg + b)
                        if l < DEPTH - 1:
                            nc.scalar.activation(
                                xT[:, g * 128:(g + 1) * 128], agg[:],
                                Relu, bias=b_sb[:])
                        else:
                            fr = sp_.tile([64, 128], f32, tag="fr")
                            nc.scalar.activation(fr[:], agg[:], Relu,
                                                 bias=b_sb[:])
                            trf = qt.tile([128, 64], f32, tag="tr")
                            nc.tensor.matmul(trf[:], fr[:],
                                             id_f32[0:64, 0:64],
                                             start=True, stop=True)
                            ot = sp_.tile([128, 64], f32, tag="ot")
                            nc.vector.tensor_copy(ot[:], trf[:])
                            nc.scalar.dma_start(
                                out_d[:, g * F:(g + 1) * F], ot[:])

    nc.compile()
    return nc


def _host_inputs(x, W, b, edge_index):
    import ml_dtypes
    deg_full = np.bincount(
        np.asarray(edge_index[1], np.int64), minlength=N
    ).astype(np.float64) + 1.0
    dinv_full = (1.0 / np.sqrt(deg_full)).astype(np.float32)
    pairs, blobs = _build_schedule(np.asarray(edge_index), dinv_full)

    ident = np.eye(128, dtype=np.float32)
    iota = np.tile(np.arange(128, dtype=np.float32)[None, :], (128, 1))
    x = np.asarray(x, np.float32)

    # slot s (pair-major) <-> xT column (2k+p)*128 + j
    s_of_col = np.empty(NP, np.int64)
    for g in range(NT):
        k, p = g // 2, g % 2
        j = np.arange(128)
        s_of_col[g * 128 + j] = 256 * k + 2 * j + p

    in_maps = []
    for c in range(NC):
        xp = np.zeros((NP, F), np.float32)
        xp[:NLOC] = x[c * NLOC:(c + 1) * NLOC]
        xt = xp[s_of_col].T                      # [64, NP]
        dg = np.ones(NP, np.float32)
        dg_loc = dinv_full[c * NLOC:(c + 1) * NLOC]
        dg[:NLOC] = dg_loc
        dinv_slot = dg                           # indexed by slot
        dinvc = np.empty((128, NT), np.float32)
        for g in range(NT):
            dinvc[:, g] = dinv_slot[s_of_col[g * 128:(g + 1) * 128]]
        m = {"xt": np.ascontiguousarray(xt).astype(ml_dtypes.bfloat16),
             "W": np.asarray(W, np.float32),
             "b": np.asarray(b, np.float32).reshape(64, 1),
             "ident": ident, "iota": iota,
             "dinvc": np.ascontiguousarray(dinvc)}
        for i, bl in enumerate(blobs[c]):
            m[f"gA{i}"] = bl["gA"]
            m[f"gB{i}"] = bl["gB"]
            m[f"qe{i}"] = bl["qe"]
            m[f"qo{i}"] = bl["qo"]
            m[f"dd{i}"] = bl["dd"]
        in_maps.append(m)
    return pairs, in_maps, s_of_col


def unpack(out_core):
    """[128, NT*F] device layout -> [NLOC, F] node-major."""
    o = out_core.reshape(128, NT, F)
    res = np.empty((NP, F), np.float32)
    j = np.arange(128)
    for g in range(NT):
        k, p = g // 2, g % 2
        res[256 * k + 2 * j + p] = o[j, g]
    return res[:NLOC]


last_results = None
last_exec_ns = None


def kernel(x, edge_index, batch_index, node_rankings, W, b, **_unused):
    import os
    import time
    from concourse.bass_utils import run_bass_kernel_spmd

    global last_results, last_exec_ns
    pairs, in_maps, _ = _host_inputs(x, W, b, np.asarray(edge_index))
    nc = _build_program(pairs)

    cores = list(range(NC))
    captured = {}
    if os.environ.get("KERNEL_TIME"):
        import jax
        orig_jit = jax.jit

        def spy_jit(*a, **kw):
            f = orig_jit(*a, **kw)

            def wrapper(*args):
                captured["fn"], captured["args"] = f, args
                return f(*args)
            return wrapper
        jax.jit = spy_jit
    try:
        if os.environ.get("KERNEL_TRACE"):
            try:
                res = run_bass_kernel_spmd(nc, in_maps, cores, trace=True)
            except Exception:
                import traceback
                traceback.print_exc()
                res = run_bass_kernel_spmd(nc, in_maps, cores)
        else:
            res = run_bass_kernel_spmd(nc, in_maps, cores)
    finally:
        if captured:
            import jax
            jax.jit = orig_jit
    if captured.get("fn") is not None:
        # warm re-execution of the captured jitted NEFF call: wall time is
        # upload + execute + sync, no retrace/compile
        import jax
        t0 = time.perf_counter()
        o = captured["fn"](*captured["args"])
        jax.block_until_ready(o)
        last_exec_ns = int((time.perf_counter() - t0) * 1e9)
    last_results = res

    out = np.empty((N, F), np.float32)
    for c in range(NC):
        out[c * NLOC:(c + 1) * NLOC] = unpack(res.results[c]["out"])
    return out



# revision 11
# speedup vs baseline: 124.8098x; 2.3017x over previous
"""GCN block v3: HBM-gather message passing with gathered one-hot selectors.

Math per layer (PyG GCNConv): x' = relu(D^-1/2 (A+I) D^-1/2 (x W) + b),
weights shared across DEPTH layers.

Layout: core c owns nodes [c*12500, (c+1)*12500); local slot s = plain local
id, tile g = s//128, lane j = s%128. y = dinv*(x@W) is stored node-major as
256B-strided rows (128 bf16, first 64 real) in DRAM:
  y_loc  [128, 12544] bf16 (SBUF mirror: partition j, tile g at elems
         [g*128, (g+1)*128)) -> AllGather -> y_full [100352, 256B rows] where
         row r = (c*128 + j)*98 + g holds node (c, s=g*128+j).

Edge phase per layer (all self-loops are ordinary edges):
  For each 128-slot slice: msg = dma_gather(y_full window, src row idx)
  [slot, 128] bf16 slot-major (no transpose, no parity); sel = dma_gather(
  table, dst slot idx) where table row j*98+g = dinv[s]*onehot(s%128) is a
  dinv-scaled identity built on device once -> one PE matmul
  agg[f, m] += msg[:, 0:64]^T @ sel accumulated per dst tile in PSUM ->
  ACT drain x' = relu(agg + b) straight into next layer's feature-major xT.

Gathers use int16 idx in 4 windows of 25088 rows; idx blobs are uploaded
un-replicated [16, S/16] and broadcast to [128, S/16] on device, staged in
DRAM, and re-loaded per layer. Per-edge upload: 2B msg idx + 2B sel idx.
"""

import numpy as np

N = 100000
F = 64
NC = 8
NLOC = 12500          # real nodes per core
NP = 12544            # padded slots per core = 98 * 128
NT = 98               # tiles per core
NWIN = 4              # gather windows over y_full rows (int16 idx limit)
WIN = 25088           # rows per window = NC * NP / NWIN
KMAX = 6144           # max gather slots per instruction
TMAX = 12             # max dst tiles per chunk (PSUM tiles in flight)
SEL_PAD = 12543       # table row with dinv=0 (slot 12543 is padding)
DEPTH = 4


def _wrap16(a):
    """[S] int16 -> [16, S/16] in the SWDGE idx order (one 16-lane replica)."""
    return np.ascontiguousarray(a.reshape(-1, 16).T)


def _build_schedule(edge_index):
    """Bucket edges + self-loops by (dst core, dst tile, src window).

    Returns (chunks, S, msg_all, sel_all):
      chunks: list of (tiles=[(g, [nsl0..nsl3])...], base=[4], k=[4]) shared
              by all cores (slot offsets/counts in the flat schedule).
      S: total slots per core.
      msg_all/sel_all: [NC, S] int16 gather indices.
    """
    src = np.asarray(edge_index[0], np.int64)
    dst = np.asarray(edge_index[1], np.int64)
    loops = np.arange(N, dtype=np.int64)
    src = np.concatenate([src, loops])
    dst = np.concatenate([dst, loops])

    c_d = dst // NLOC
    s_d = dst - c_d * NLOC
    g_d = s_d >> 7
    c_s = src // NLOC
    s_s = src - c_s * NLOC
    grow = (c_s * 128 + (s_s & 127)) * NT + (s_s >> 7)
    w = grow // WIN
    rel = (grow - w * WIN).astype(np.int16)
    selidx = ((s_d & 127) * NT + g_d).astype(np.int16)

    key = (c_d * NT + g_d) * NWIN + w
    cnt = np.bincount(key, minlength=NC * NT * NWIN).reshape(NC, NT, NWIN)
    gmax = ((cnt.max(axis=0) + 127) // 128) * 128      # [NT, NWIN]
    nsl = gmax >> 7

    # chunk tiles: per-window slices <= KMAX/128, tile count <= TMAX
    chunks = []
    cur, acc = [], np.zeros(NWIN, np.int64)
    for g in range(NT):
        n = nsl[g]
        if cur and (np.any(acc + n > KMAX // 128) or len(cur) >= TMAX):
            chunks.append(cur)
            cur, acc = [], np.zeros(NWIN, np.int64)
        cur.append(g)
        acc = acc + n
    if cur:
        chunks.append(cur)

    # slot layout: chunk-major, window-major, tile-major
    base_gw = np.zeros((NT, NWIN), np.int64)
    meta = []
    off = 0
    for tiles in chunks:
        base = []
        kk = []
        for ww in range(NWIN):
            base.append(off)
            for g in tiles:
                base_gw[g, ww] = off
                off += int(gmax[g, ww])
            kk.append(off - base[-1])
        meta.append((
            [(g, [int(nsl[g, ww]) for ww in range(NWIN)]) for g in tiles],
            base, kk))
    S = off
    assert S % 128 == 0

    order = np.argsort(key, kind="stable")
    ks = key[order]
    uniq, starts, counts_u = np.unique(ks, return_index=True,
                                       return_counts=True)
    rank = np.arange(len(ks)) - np.repeat(starts, counts_u)
    pos = base_gw[g_d[order], w[order]] + rank
    core = c_d[order]

    msg_all = np.zeros((NC, S), np.int16)
    sel_all = np.full((NC, S), SEL_PAD, np.int16)
    msg_all[core, pos] = rel[order]
    sel_all[core, pos] = selidx[order]
    return meta, S, msg_all, sel_all


def _build_program(chunks, S):
    from concourse import bacc, tile
    from concourse import mybir

    f32, i16, bf16 = mybir.dt.float32, mybir.dt.int16, mybir.dt.bfloat16
    nc = bacc.Bacc("TRN2", target_bir_lowering=False, debug=False,
                   num_devices=NC, num_swdge_queues=4)

    xt_in = nc.dram_tensor("xt", [64, NP], bf16, kind="ExternalInput")
    w_in = nc.dram_tensor("W", [F, F], bf16, kind="ExternalInput")
    b_in = nc.dram_tensor("b", [64, 1], f32, kind="ExternalInput")
    id_in = nc.dram_tensor("ident", [128, 128], bf16, kind="ExternalInput")
    dinv_in = nc.dram_tensor("dinvc", [128, NT], f32, kind="ExternalInput")
    gm_in = nc.dram_tensor("gm", [16, S // 16], i16, kind="ExternalInput")
    gs_in = nc.dram_tensor("gs", [16, S // 16], i16, kind="ExternalInput")
    out_d = nc.dram_tensor("out", [64, NP], f32, kind="ExternalOutput")

    y_loc = [nc.dram_tensor(f"y_loc{i}", [128, NP], bf16) for i in range(2)]
    y_full = [nc.dram_tensor(f"y_full{i}", [NC * NP, 128], bf16,
                             addr_space="Shared") for i in range(2)]
    table_d = nc.dram_tensor("table", [NP, 128], bf16)
    gm_rep = nc.dram_tensor("gm_rep", [128, S // 16], i16)
    gs_rep = nc.dram_tensor("gs_rep", [128, S // 16], i16)

    Copy = mybir.ActivationFunctionType.Copy
    Relu = mybir.ActivationFunctionType.Relu
    mult = mybir.AluOpType.mult

    with tile.TileContext(nc) as tc:
        with tc.tile_pool(name="persist", bufs=1) as pp, \
             tc.tile_pool(name="slab", bufs=1) as bp, \
             tc.tile_pool(name="idx", bufs=6) as ip, \
             tc.tile_pool(name="msg", bufs=3) as mp, \
             tc.tile_pool(name="sel", bufs=3) as sp_, \
             tc.tile_pool(name="outs", bufs=3) as op_, \
             tc.tile_pool(name="ph", bufs=2, space="PSUM") as qm, \
             tc.tile_pool(name="pagg", bufs=6, space="PSUM") as qa:

            xT = pp.tile([64, NP], bf16)
            ystage = pp.tile([128, NP], bf16)
            tstage = pp.tile([128, NP], bf16)
            w_sb = pp.tile([F, F], bf16)
            b_sb = pp.tile([64, 1], f32)
            id_sb = pp.tile([128, 128], bf16)
            dinvc = pp.tile([128, NT], f32)
            zt = pp.tile([128, 512], bf16)
            nc.vector.memset(zt[:], 0.0)

            nc.sync.dma_start(w_sb[:], w_in[:])
            nc.sync.dma_start(b_sb[:], b_in[:])
            nc.sync.dma_start(id_sb[:], id_in[:])
            nc.sync.dma_start(dinvc[:], dinv_in[:])
            nc.sync.dma_start(xT[:], xt_in[:])

            # broadcast [16, S/16] idx blobs to [128, S/16] and stage in DRAM
            for src_t, dst_t in ((gm_in, gm_rep), (gs_in, gs_rep)):
                slab = bp.tile([128, S // 16], i16, tag="slab")
                nc.sync.dma_start(slab[0:16, :], src_t[:])
                nc.sync.dma_start(slab[16:32, :], slab[0:16, :])
                nc.sync.dma_start(slab[32:64, :], slab[0:32, :])
                nc.sync.dma_start(slab[64:128, :], slab[0:64, :])
                nc.sync.dma_start(dst_t[:], slab[:])

            # dinv-scaled identity table: row j*98+g = dinv[g*128+j]*onehot(j)
            for g in range(NT):
                nc.vector.tensor_scalar(
                    tstage[:, g * 128:(g + 1) * 128], id_sb[:],
                    dinvc[:, g:g + 1], None, mult)
            nc.sync.dma_start(table_d[:], tstage[:])

            qrr = [0]

            def next_q():
                qrr[0] = (qrr[0] + 1) % 4
                return qrr[0]

            for l in range(DEPTH):
                yf = y_full[l % 2]
                yl = y_loc[l % 2]
                with tc.nc.named_scope(f"L{l}_y"):
                    for g in range(NT):
                        h = qm.tile([128, F], f32, tag="h")
                        nc.tensor.matmul(h[:], xT[:, g * 128:(g + 1) * 128],
                                         w_sb[:], start=True, stop=True)
                        nc.scalar.activation(
                            ystage[:, g * 128:g * 128 + F], h[:], Copy,
                            scale=dinvc[:, g:g + 1])
                    nc.sync.dma_start(yl[:], ystage[:])
                with tc.nc.named_scope(f"L{l}_ag"):
                    nc.gpsimd.collective_compute(
                        "AllGather", mybir.AluOpType.bypass,
                        replica_groups=[list(range(NC))],
                        ins=[yl[:]], outs=[yf[:]])
                with tc.nc.named_scope(f"L{l}_edge"):
                    for tiles, base, kk in chunks:
                        # 4 agg accumulators share one 2KB PSUM bank tile
                        nbank = (len(tiles) + 3) // 4
                        banks = [qa.tile([64, 512], f32, tag="agg",
                                         name="agg") for _ in range(nbank)]
                        # start=True clears has_written for the whole bank, so
                        # open each bank once with a zeroing matmul and have
                        # every real matmul accumulate (start=False)
                        for bk in banks:
                            nc.tensor.matmul(bk[:], zt[:, 0:64], zt[:],
                                             start=True, stop=False)
                        aggs = {}
                        done = {}
                        total = {}
                        for i, (g, nsl) in enumerate(tiles):
                            aggs[g] = banks[i // 4][:, (i % 4) * 128:
                                                    (i % 4 + 1) * 128]
                            done[g] = 0
                            total[g] = sum(nsl)
                        for ww in range(NWIN):
                            k = kk[ww]
                            if k == 0:
                                continue
                            b0 = base[ww]
                            gmt = ip.tile([128, KMAX // 16], i16, tag="gm")
                            nc.sync.dma_start(
                                gmt[:, :k // 16],
                                gm_rep[:, b0 // 16:(b0 + k) // 16])
                            gst = ip.tile([128, KMAX // 16], i16, tag="gs")
                            nc.sync.dma_start(
                                gst[:, :k // 16],
                                gs_rep[:, b0 // 16:(b0 + k) // 16])
                            msg = mp.tile([128, KMAX // 128, 128], bf16,
                                          tag="msg")
                            nc.gpsimd.dma_gather(
                                msg[:, :k // 128, :],
                                yf[ww * WIN:(ww + 1) * WIN, :],
                                gmt[:, :k // 16], k, k, 128,
                                transpose=False, single_packet=False,
                                queue_num=next_q())
                            sel = sp_.tile([128, KMAX // 128, 128], bf16,
                                           tag="sel")
                            nc.gpsimd.dma_gather(
                                sel[:, :k // 128, :], table_d[:],
                                gst[:, :k // 16], k, k, 128,
                                transpose=False, single_packet=False,
                                queue_num=next_q())
                            off = 0
                            for g, nsl in tiles:
                                for u in range(nsl[ww]):
                                    done[g] += 1
                                    nc.tensor.matmul(
                                        aggs[g][:],
                                        msg[:, off + u, 0:64],
                                        sel[:, off + u, :],
                                        start=False,
                                        stop=(done[g] == total[g]))
                                off += nsl[ww]
                        for g, nsl in tiles:
                            if l < DEPTH - 1:
                                nc.scalar.activation(
                                    xT[:, g * 128:(g + 1) * 128],
                                    aggs[g][:], Relu, bias=b_sb[:])
                            else:
                                fr = op_.tile([64, 128], f32, tag="fr")
                                nc.scalar.activation(fr[:], aggs[g][:],
                                                     Relu, bias=b_sb[:])
                                nc.scalar.dma_start(
                                    out_d[:, g * 128:(g + 1) * 128], fr[:])

    nc.compile()
    return nc


def _host_inputs(x, W, b, edge_index):
    import ml_dtypes
    deg = np.bincount(np.asarray(edge_index[1], np.int64),
                      minlength=N).astype(np.float64) + 1.0
    dinv_full = (1.0 / np.sqrt(deg)).astype(np.float32)
    chunks, S, msg_all, sel_all = _build_schedule(np.asarray(edge_index))

    x = np.asarray(x, np.float32)
    in_maps = []
    for c in range(NC):
        xp = np.zeros((NP, F), np.float32)
        xp[:NLOC] = x[c * NLOC:(c + 1) * NLOC]
        dinv_slot = np.zeros(NP, np.float32)
        dinv_slot[:NLOC] = dinv_full[c * NLOC:(c + 1) * NLOC]
        m = {"xt": np.ascontiguousarray(xp.T).astype(ml_dtypes.bfloat16),
             "W": np.asarray(W, np.float32).astype(ml_dtypes.bfloat16),
             "b": np.asarray(b, np.float32).reshape(64, 1),
             "ident": np.eye(128, dtype=ml_dtypes.bfloat16),
             "dinvc": np.ascontiguousarray(dinv_slot.reshape(NT, 128).T),
             "gm": _wrap16(msg_all[c]),
             "gs": _wrap16(sel_all[c])}
        in_maps.append(m)
    return chunks, S, in_maps


last_results = None
last_exec_ns = None


def kernel(x, edge_index, batch_index, node_rankings, W, b, **_unused):
    import os
    import time
    from concourse.bass_utils import run_bass_kernel_spmd

    global last_results, last_exec_ns
    chunks, S, in_maps = _host_inputs(x, W, b, np.asarray(edge_index))
    nc = _build_program(chunks, S)

    cores = list(range(NC))
    captured = {}
    if os.environ.get("KERNEL_TIME"):
        import jax
        orig_jit = jax.jit

        def spy_jit(*a, **kw):
            f = orig_jit(*a, **kw)

            def wrapper(*args):
                captured["fn"], captured["args"] = f, args
                return f(*args)
            return wrapper
        jax.jit = spy_jit
    try:
        if os.environ.get("KERNEL_TRACE"):
            try:
                res = run_bass_kernel_spmd(nc, in_maps, cores, trace=True)
            except Exception:
                import traceback
                traceback.print_exc()
                res = run_bass_kernel_spmd(nc, in_maps, cores)
        else:
            res = run_bass_kernel_spmd(nc, in_maps, cores)
    finally:
        if captured:
            import jax
            jax.jit = orig_jit
    if captured.get("fn") is not None:
        # warm re-execution of the captured jitted NEFF call: wall time is
        # upload + execute + sync, no retrace/compile
        import jax
        t0 = time.perf_counter()
        o = captured["fn"](*captured["args"])
        jax.block_until_ready(o)
        last_exec_ns = int((time.perf_counter() - t0) * 1e9)
    last_results = res

    out = np.empty((N, F), np.float32)
    for c in range(NC):
        out[c * NLOC:(c + 1) * NLOC] = res.results[c]["out"].T[:NLOC]
    return out


# revision 18
# speedup vs baseline: 177.8652x; 1.4251x over previous
"""GCN block v3: HBM-gather message passing with gathered one-hot selectors.

Math per layer (PyG GCNConv): x' = relu(D^-1/2 (A+I) D^-1/2 (x W) + b),
weights shared across DEPTH layers.

Layout: core c owns nodes [c*12500, (c+1)*12500); local slot s = plain local
id, tile g = s//128, lane j = s%128. y = dinv*(x@W) is stored node-major as
256B-strided rows (128 bf16, first 64 real) in DRAM:
  y_loc  [128, 12544] bf16 (SBUF mirror: partition j, tile g at elems
         [g*128, (g+1)*128)) -> AllGather -> y_full [100352, 256B rows] where
         row r = (c*128 + j)*98 + g holds node (c, s=g*128+j).

Edge phase per layer (all self-loops are ordinary edges):
  For each 128-slot slice: msg = dma_gather(y_full window, src row idx)
  [slot, 128] bf16 slot-major (no transpose, no parity); sel = dma_gather(
  table, dst slot idx) where table row j*98+g = dinv[s]*onehot(s%128) is a
  dinv-scaled identity built on device once -> one PE matmul
  agg[f, m] += msg[:, 0:64]^T @ sel accumulated per dst tile in PSUM ->
  ACT drain x' = relu(agg + b) straight into next layer's feature-major xT.

Gathers use int16 idx in 4 windows of 25088 rows; idx blobs are uploaded
un-replicated [16, S/16] and broadcast to [128, S/16] on device, staged in
DRAM, and re-loaded per layer. Per-edge upload: 2B msg idx + 2B sel idx.
"""

import numpy as np

N = 100000
F = 64
NC = 8
NLOC = 12500          # real nodes per core
NP = 12544            # padded slots per core = 98 * 128
NT = 98               # tiles per core
NWIN = 4              # gather windows over y_full rows (int16 idx limit)
WIN = 25088           # rows per window = NC * NP / NWIN
KMAX = 6144           # max gather slots per instruction
TMAX = 12             # max dst tiles per chunk (PSUM tiles in flight)
SEL_PAD = 12543       # table row with dinv=0 (slot 12543 is padding)
DEPTH = 4


def _wrap16(a):
    """[S] int16 -> [16, S/16] in the SWDGE idx order (one 16-lane replica)."""
    return np.ascontiguousarray(a.reshape(-1, 16).T)


def _build_schedule(edge_index):
    """Bucket edges + self-loops by (dst core, dst tile, src window).

    Returns (chunks, S, msg_all, sel_all):
      chunks: list of (tiles=[(g, [nsl0..nsl3])...], base=[4], k=[4]) shared
              by all cores (slot offsets/counts in the flat schedule).
      S: total slots per core.
      msg_all/sel_all: [NC, S] int16 gather indices.
    """
    src = np.asarray(edge_index[0], np.int64)
    dst = np.asarray(edge_index[1], np.int64)
    # self-loops are NOT scheduled: they are applied as one direct matmul per
    # dst tile (y_stage tile against the dinv-scaled identity)

    c_d = dst // NLOC
    s_d = dst - c_d * NLOC
    g_d = s_d >> 7
    c_s = src // NLOC
    s_s = src - c_s * NLOC
    grow = (c_s * 128 + (s_s & 127)) * NT + (s_s >> 7)
    w = grow // WIN
    rel = (grow - w * WIN).astype(np.int16)
    selidx = ((s_d & 127) * NT + g_d).astype(np.int16)

    key = (c_d * NT + g_d) * NWIN + w
    cnt = np.bincount(key, minlength=NC * NT * NWIN).reshape(NC, NT, NWIN)
    gmax = ((cnt.max(axis=0) + 127) // 128) * 128      # [NT, NWIN]
    nsl = gmax >> 7

    # chunk tiles: per-window slices <= KMAX/128, tile count <= TMAX
    chunks = []
    cur, acc = [], np.zeros(NWIN, np.int64)
    for g in range(NT):
        n = nsl[g]
        if cur and (np.any(acc + n > KMAX // 128) or len(cur) >= TMAX):
            chunks.append(cur)
            cur, acc = [], np.zeros(NWIN, np.int64)
        cur.append(g)
        acc = acc + n
    if cur:
        chunks.append(cur)

    # slot layout: chunk-major, window-major, tile-major
    base_gw = np.zeros((NT, NWIN), np.int64)
    meta = []
    off = 0
    for tiles in chunks:
        base = []
        kk = []
        for ww in range(NWIN):
            base.append(off)
            for g in tiles:
                base_gw[g, ww] = off
                off += int(gmax[g, ww])
            kk.append(off - base[-1])
        meta.append((
            [(g, [int(nsl[g, ww]) for ww in range(NWIN)]) for g in tiles],
            base, kk))
    S = off
    assert S % 128 == 0

    order = np.argsort(key, kind="stable")
    ks = key[order]
    uniq, starts, counts_u = np.unique(ks, return_index=True,
                                       return_counts=True)
    rank = np.arange(len(ks)) - np.repeat(starts, counts_u)
    pos = base_gw[g_d[order], w[order]] + rank
    core = c_d[order]

    msg_all = np.zeros((NC, S), np.int16)
    sel_all = np.full((NC, S), SEL_PAD, np.int16)
    msg_all[core, pos] = rel[order]
    sel_all[core, pos] = selidx[order]
    return meta, S, msg_all, sel_all


def _build_program(chunks, S):
    from concourse import bacc, tile
    from concourse import mybir

    f32, i16, bf16 = mybir.dt.float32, mybir.dt.int16, mybir.dt.bfloat16
    nc = bacc.Bacc("TRN2", target_bir_lowering=False, debug=False,
                   num_devices=NC, num_swdge_queues=4)

    xt_in = nc.dram_tensor("xt", [64, NP], bf16, kind="ExternalInput")
    w_in = nc.dram_tensor("W", [F, F], bf16, kind="ExternalInput")
    b_in = nc.dram_tensor("b", [64, 1], f32, kind="ExternalInput")
    id_in = nc.dram_tensor("ident", [128, 128], bf16, kind="ExternalInput")
    dinv_in = nc.dram_tensor("dinvc", [128, NT], f32, kind="ExternalInput")
    gm_in = nc.dram_tensor("gm", [16, S // 16], i16, kind="ExternalInput")
    gs_in = nc.dram_tensor("gs", [16, S // 16], i16, kind="ExternalInput")
    out_d = nc.dram_tensor("out", [64, NP], f32, kind="ExternalOutput")

    y_loc = [nc.dram_tensor(f"y_loc{i}", [128, NP], bf16) for i in range(2)]
    y_full = [nc.dram_tensor(f"y_full{i}", [NC * NP, 128], bf16,
                             addr_space="Shared") for i in range(2)]
    table_d = nc.dram_tensor("table", [NP, 128], bf16)
    gm_rep = nc.dram_tensor("gm_rep", [128, S // 16], i16)
    gs_rep = nc.dram_tensor("gs_rep", [128, S // 16], i16)

    Copy = mybir.ActivationFunctionType.Copy
    Relu = mybir.ActivationFunctionType.Relu
    mult = mybir.AluOpType.mult

    with tile.TileContext(nc) as tc:
        with tc.tile_pool(name="persist", bufs=1) as pp, \
             tc.tile_pool(name="idx", bufs=8) as ip, \
             tc.tile_pool(name="msg", bufs=4) as mp, \
             tc.tile_pool(name="sel", bufs=4) as sp_, \
             tc.tile_pool(name="outs", bufs=3) as op_, \
             tc.tile_pool(name="ph", bufs=2, space="PSUM") as qm, \
             tc.tile_pool(name="pagg", bufs=6, space="PSUM") as qa:

            xT = pp.tile([64, NP], bf16)
            ystage = pp.tile([128, NP], bf16)
            tstage = pp.tile([128, NP], bf16)
            w_sb = pp.tile([F, F], bf16)
            b_sb = pp.tile([64, 1], f32)
            id_sb = pp.tile([128, 128], bf16)
            dinvc = pp.tile([128, NT], f32)
            zt = pp.tile([128, 512], bf16)
            nc.vector.memset(zt[:], 0.0)

            nc.sync.dma_start(w_sb[:], w_in[:])
            nc.sync.dma_start(b_sb[:], b_in[:])
            nc.sync.dma_start(id_sb[:], id_in[:])
            nc.sync.dma_start(dinvc[:], dinv_in[:])
            nc.sync.dma_start(xT[:], xt_in[:])

            # broadcast [16, S/16] idx blobs to [128, S/16] DRAM-to-DRAM
            for src_t, dst_t in ((gm_in, gm_rep), (gs_in, gs_rep)):
                nc.sync.dma_start(dst_t[0:16, :], src_t[:])
                nc.sync.dma_start(dst_t[16:32, :], dst_t[0:16, :])
                nc.sync.dma_start(dst_t[32:64, :], dst_t[0:32, :])
                nc.sync.dma_start(dst_t[64:128, :], dst_t[0:64, :])

            # dinv-scaled identity table: row j*98+g = dinv[g*128+j]*onehot(j)
            for g in range(NT):
                nc.vector.tensor_scalar(
                    tstage[:, g * 128:(g + 1) * 128], id_sb[:],
                    dinvc[:, g:g + 1], None, mult)
            nc.sync.dma_start(table_d[:], tstage[:])

            qrr = [0]

            def next_q():
                qrr[0] = (qrr[0] + 1) % 4
                return qrr[0]

            for l in range(DEPTH):
                yf = y_full[l % 2]
                yl = y_loc[l % 2]
                with tc.nc.named_scope(f"L{l}_y"):
                    for g in range(NT):
                        h = qm.tile([128, F], f32, tag="h")
                        nc.tensor.matmul(h[:], xT[:, g * 128:(g + 1) * 128],
                                         w_sb[:], start=True, stop=True)
                        nc.scalar.activation(
                            ystage[:, g * 128:g * 128 + F], h[:], Copy,
                            scale=dinvc[:, g:g + 1])
                    nc.sync.dma_start(yl[:], ystage[:])
                with tc.nc.named_scope(f"L{l}_ag"):
                    nc.gpsimd.collective_compute(
                        "AllGather", mybir.AluOpType.bypass,
                        replica_groups=[list(range(NC))],
                        ins=[yl[:]], outs=[yf[:]])
                with tc.nc.named_scope(f"L{l}_edge"):
                    for tiles, base, kk in chunks:
                        # 4 agg accumulators share one 2KB PSUM bank tile
                        nbank = (len(tiles) + 3) // 4
                        banks = [qa.tile([64, 512], f32, tag="agg",
                                         name="agg") for _ in range(nbank)]
                        # start=True clears has_written for the whole bank, so
                        # open each bank once with a zeroing matmul and have
                        # every real matmul accumulate (start=False)
                        for bk in banks:
                            nc.tensor.matmul(bk[:], zt[:, 0:64], zt[:],
                                             start=True, stop=False)
                        aggs = {}
                        done = {}
                        total = {}
                        for i, (g, nsl) in enumerate(tiles):
                            aggs[g] = banks[i // 4][:, (i % 4) * 128:
                                                    (i % 4 + 1) * 128]
                            done[g] = 1
                            total[g] = sum(nsl) + 1
                            # self-loop: y_tile^T @ (dinv-scaled identity)
                            nc.tensor.matmul(
                                aggs[g][:],
                                ystage[:, g * 128:g * 128 + F],
                                tstage[:, g * 128:(g + 1) * 128],
                                start=False, stop=(done[g] == total[g]))
                        for ww in range(NWIN):
                            k = kk[ww]
                            if k == 0:
                                continue
                            b0 = base[ww]
                            gmt = ip.tile([128, KMAX // 16], i16, tag="gm")
                            nc.sync.dma_start(
                                gmt[:, :k // 16],
                                gm_rep[:, b0 // 16:(b0 + k) // 16])
                            gst = ip.tile([128, KMAX // 16], i16, tag="gs")
                            nc.sync.dma_start(
                                gst[:, :k // 16],
                                gs_rep[:, b0 // 16:(b0 + k) // 16])
                            msg = mp.tile([128, KMAX // 128, 128], bf16,
                                          tag="msg")
                            nc.gpsimd.dma_gather(
                                msg[:, :k // 128, :],
                                yf[ww * WIN:(ww + 1) * WIN, :],
                                gmt[:, :k // 16], k, k, 128,
                                transpose=False, single_packet=False,
                                queue_num=next_q())
                            sel = sp_.tile([128, KMAX // 128, 128], bf16,
                                           tag="sel")
                            nc.gpsimd.dma_gather(
                                sel[:, :k // 128, :], table_d[:],
                                gst[:, :k // 16], k, k, 128,
                                transpose=False, single_packet=False,
                                queue_num=next_q())
                            off = 0
                            for g, nsl in tiles:
                                for u in range(nsl[ww]):
                                    done[g] += 1
                                    nc.tensor.matmul(
                                        aggs[g][:],
                                        msg[:, off + u, 0:64],
                                        sel[:, off + u, :],
                                        start=False,
                                        stop=(done[g] == total[g]))
                                off += nsl[ww]
                        for g, nsl in tiles:
                            if l < DEPTH - 1:
                                nc.scalar.activation(
                                    xT[:, g * 128:(g + 1) * 128],
                                    aggs[g][:], Relu, bias=b_sb[:])
                            else:
                                fr = op_.tile([64, 128], f32, tag="fr")
                                nc.scalar.activation(fr[:], aggs[g][:],
                                                     Relu, bias=b_sb[:])
                                nc.scalar.dma_start(
                                    out_d[:, g * 128:(g + 1) * 128], fr[:])

    nc.compile()
    return nc


def _host_inputs(x, W, b, edge_index):
    import ml_dtypes
    deg = np.bincount(np.asarray(edge_index[1], np.int64),
                      minlength=N).astype(np.float64) + 1.0
    dinv_full = (1.0 / np.sqrt(deg)).astype(np.float32)
    chunks, S, msg_all, sel_all = _build_schedule(np.asarray(edge_index))

    x = np.asarray(x, np.float32)
    in_maps = []
    for c in range(NC):
        xp = np.zeros((NP, F), np.float32)
        xp[:NLOC] = x[c * NLOC:(c + 1) * NLOC]
        dinv_slot = np.zeros(NP, np.float32)
        dinv_slot[:NLOC] = dinv_full[c * NLOC:(c + 1) * NLOC]
        m = {"xt": np.ascontiguousarray(xp.T).astype(ml_dtypes.bfloat16),
             "W": np.asarray(W, np.float32).astype(ml_dtypes.bfloat16),
             "b": np.asarray(b, np.float32).reshape(64, 1),
             "ident": np.eye(128, dtype=ml_dtypes.bfloat16),
             "dinvc": np.ascontiguousarray(dinv_slot.reshape(NT, 128).T),
             "gm": _wrap16(msg_all[c]),
             "gs": _wrap16(sel_all[c])}
        in_maps.append(m)
    return chunks, S, in_maps


last_results = None
last_exec_ns = None


def kernel(x, edge_index, batch_index, node_rankings, W, b, **_unused):
    import os
    import time
    from concourse.bass_utils import run_bass_kernel_spmd

    global last_results, last_exec_ns
    chunks, S, in_maps = _host_inputs(x, W, b, np.asarray(edge_index))
    nc = _build_program(chunks, S)

    cores = list(range(NC))
    captured = {}
    if os.environ.get("KERNEL_TIME"):
        import jax
        orig_jit = jax.jit

        def spy_jit(*a, **kw):
            f = orig_jit(*a, **kw)

            def wrapper(*args):
                captured["fn"], captured["args"] = f, args
                return f(*args)
            return wrapper
        jax.jit = spy_jit
    try:
        if os.environ.get("KERNEL_TRACE"):
            try:
                res = run_bass_kernel_spmd(nc, in_maps, cores, trace=True)
            except Exception:
                import traceback
                traceback.print_exc()
                res = run_bass_kernel_spmd(nc, in_maps, cores)
        else:
            res = run_bass_kernel_spmd(nc, in_maps, cores)
    finally:
        if captured:
            import jax
            jax.jit = orig_jit
    if captured.get("fn") is not None:
        # warm re-execution of the captured jitted NEFF call: wall time is
        # upload + execute + sync, no retrace/compile
        import jax
        t0 = time.perf_counter()
        o = captured["fn"](*captured["args"])
        jax.block_until_ready(o)
        last_exec_ns = int((time.perf_counter() - t0) * 1e9)
    last_results = res

    out = np.empty((N, F), np.float32)
    for c in range(NC):
        out[c * NLOC:(c + 1) * NLOC] = res.results[c]["out"].T[:NLOC]
    return out


# revision 22
# speedup vs baseline: 197.1132x; 1.1082x over previous
"""GCN block v3: HBM-gather message passing with gathered one-hot selectors.

Math per layer (PyG GCNConv): x' = relu(D^-1/2 (A+I) D^-1/2 (x W) + b),
weights shared across DEPTH layers.

Layout: core c owns nodes [c*12500, (c+1)*12500); local slot s = plain local
id, tile g = s//128, lane j = s%128. y = dinv*(x@W) is stored node-major as
256B-strided rows (128 bf16, first 64 real) in DRAM:
  y_loc  [128, 12544] bf16 (SBUF mirror: partition j, tile g at elems
         [g*128, (g+1)*128)) -> AllGather -> y_full [100352, 256B rows] where
         row r = (c*128 + j)*98 + g holds node (c, s=g*128+j).

Edge phase per layer (all self-loops are ordinary edges):
  For each 128-slot slice: msg = dma_gather(y_full window, src row idx)
  [slot, 128] bf16 slot-major (no transpose, no parity); sel = dma_gather(
  table, dst slot idx) where table row j*98+g = dinv[s]*onehot(s%128) is a
  dinv-scaled identity built on device once -> one PE matmul
  agg[f, m] += msg[:, 0:64]^T @ sel accumulated per dst tile in PSUM ->
  ACT drain x' = relu(agg + b) straight into next layer's feature-major xT.

Gathers use int16 idx in 4 windows of 25088 rows; idx blobs are uploaded
un-replicated [16, S/16] and broadcast to [128, S/16] on device, staged in
DRAM, and re-loaded per layer. Per-edge upload: 2B msg idx + 2B sel idx.
"""

import numpy as np

N = 100000
F = 64
NC = 8
NLOC = 12500          # real nodes per core
NP = 12544            # padded slots per core = 98 * 128
NT = 98               # tiles per core
NWIN = 4              # gather windows over y_full rows (int16 idx limit)
WIN = 25088           # rows per window = NC * NP / NWIN
KMAX = 3072           # max gather slots per instruction
TMAX = 12             # max dst tiles per chunk (PSUM tiles in flight)
SEL_PAD = 12543       # table row with dinv=0 (slot 12543 is padding)
DEPTH = 4


def _wrap16(a):
    """[S] int16 -> [16, S/16] in the SWDGE idx order (one 16-lane replica)."""
    return np.ascontiguousarray(a.reshape(-1, 16).T)


def _build_schedule(edge_index):
    """Bucket edges + self-loops by (dst core, dst tile, src window).

    Returns (chunks, S, msg_all, sel_all):
      chunks: list of (tiles=[(g, [nsl0..nsl3])...], base=[4], k=[4]) shared
              by all cores (slot offsets/counts in the flat schedule).
      S: total slots per core.
      msg_all/sel_all: [NC, S] int16 gather indices.
    """
    src = np.asarray(edge_index[0], np.int64)
    dst = np.asarray(edge_index[1], np.int64)
    # self-loops are NOT scheduled: they are applied as one direct matmul per
    # dst tile (y_stage tile against the dinv-scaled identity)

    c_d = dst // NLOC
    s_d = dst - c_d * NLOC
    g_d = s_d >> 7
    c_s = src // NLOC
    s_s = src - c_s * NLOC
    grow = (c_s * 128 + (s_s & 127)) * NT + (s_s >> 7)
    w = grow // WIN
    rel = (grow - w * WIN).astype(np.int16)
    selidx = ((s_d & 127) * NT + g_d).astype(np.int16)

    key = (c_d * NT + g_d) * NWIN + w
    cnt = np.bincount(key, minlength=NC * NT * NWIN).reshape(NC, NT, NWIN)
    gmax = ((cnt.max(axis=0) + 127) // 128) * 128      # [NT, NWIN]
    nsl = gmax >> 7

    # chunk tiles: per-window slices <= KMAX/128, tile count <= TMAX
    chunks = []
    cur, acc = [], np.zeros(NWIN, np.int64)
    for g in range(NT):
        n = nsl[g]
        if cur and (np.any(acc + n > KMAX // 128) or len(cur) >= TMAX):
            chunks.append(cur)
            cur, acc = [], np.zeros(NWIN, np.int64)
        cur.append(g)
        acc = acc + n
    if cur:
        chunks.append(cur)

    # slot layout: chunk-major, window-major, tile-major
    base_gw = np.zeros((NT, NWIN), np.int64)
    meta = []
    off = 0
    for tiles in chunks:
        base = []
        kk = []
        for ww in range(NWIN):
            base.append(off)
            for g in tiles:
                base_gw[g, ww] = off
                off += int(gmax[g, ww])
            kk.append(off - base[-1])
        meta.append((
            [(g, [int(nsl[g, ww]) for ww in range(NWIN)]) for g in tiles],
            base, kk))
    S = off
    assert S % 128 == 0

    order = np.argsort(key, kind="stable")
    ks = key[order]
    uniq, starts, counts_u = np.unique(ks, return_index=True,
                                       return_counts=True)
    rank = np.arange(len(ks)) - np.repeat(starts, counts_u)
    pos = base_gw[g_d[order], w[order]] + rank
    core = c_d[order]

    msg_all = np.zeros((NC, S), np.int16)
    sel_all = np.full((NC, S), SEL_PAD, np.int16)
    msg_all[core, pos] = rel[order]
    sel_all[core, pos] = selidx[order]
    return meta, S, msg_all, sel_all


def _build_program(chunks, S):
    from concourse import bacc, tile
    from concourse import mybir

    f32, i16, bf16 = mybir.dt.float32, mybir.dt.int16, mybir.dt.bfloat16
    nc = bacc.Bacc("TRN2", target_bir_lowering=False, debug=False,
                   num_devices=NC, num_swdge_queues=4)

    xt_in = nc.dram_tensor("xt", [64, NP], bf16, kind="ExternalInput")
    w_in = nc.dram_tensor("W", [F, F], bf16, kind="ExternalInput")
    b_in = nc.dram_tensor("b", [64, 1], f32, kind="ExternalInput")
    id_in = nc.dram_tensor("ident", [128, 128], bf16, kind="ExternalInput")
    dinv_in = nc.dram_tensor("dinvc", [128, NT], f32, kind="ExternalInput")
    gm_in = nc.dram_tensor("gm", [16, S // 16], i16, kind="ExternalInput")
    gs_in = nc.dram_tensor("gs", [16, S // 16], i16, kind="ExternalInput")
    out_d = nc.dram_tensor("out", [64, NP], f32, kind="ExternalOutput")

    y_loc = [nc.dram_tensor(f"y_loc{i}", [128, NP], bf16) for i in range(2)]
    y_full = [nc.dram_tensor(f"y_full{i}", [NC * NP, 128], bf16,
                             addr_space="Shared") for i in range(2)]
    table_d = nc.dram_tensor("table", [NP, 128], bf16)
    sel_cache = nc.dram_tensor("sel_cache", [S, 128], bf16)
    gm_rep = nc.dram_tensor("gm_rep", [128, S // 16], i16)
    gs_rep = nc.dram_tensor("gs_rep", [128, S // 16], i16)

    Copy = mybir.ActivationFunctionType.Copy
    Relu = mybir.ActivationFunctionType.Relu
    mult = mybir.AluOpType.mult

    with tile.TileContext(nc) as tc:
        with tc.tile_pool(name="persist", bufs=1) as pp, \
             tc.tile_pool(name="idx", bufs=8) as ip, \
             tc.tile_pool(name="msg", bufs=6) as mp, \
             tc.tile_pool(name="sel", bufs=6) as sp_, \
             tc.tile_pool(name="outs", bufs=3) as op_, \
             tc.tile_pool(name="ph", bufs=2, space="PSUM") as qm, \
             tc.tile_pool(name="pagg", bufs=6, space="PSUM") as qa:

            xT = pp.tile([64, NP], bf16)
            ystage = pp.tile([128, NP], bf16)
            tstage = pp.tile([128, NP], bf16)
            w_sb = pp.tile([F, F], bf16)
            b_sb = pp.tile([64, 1], f32)
            id_sb = pp.tile([128, 128], bf16)
            dinvc = pp.tile([128, NT], f32)
            zt = pp.tile([128, 512], bf16)
            nc.vector.memset(zt[:], 0.0)

            nc.sync.dma_start(w_sb[:], w_in[:])
            nc.sync.dma_start(b_sb[:], b_in[:])
            nc.sync.dma_start(id_sb[:], id_in[:])
            nc.sync.dma_start(dinvc[:], dinv_in[:])
            nc.sync.dma_start(xT[:], xt_in[:])

            # broadcast [16, S/16] idx blobs to [128, S/16] DRAM-to-DRAM
            for src_t, dst_t in ((gm_in, gm_rep), (gs_in, gs_rep)):
                nc.sync.dma_start(dst_t[0:16, :], src_t[:])
                nc.sync.dma_start(dst_t[16:32, :], dst_t[0:16, :])
                nc.sync.dma_start(dst_t[32:64, :], dst_t[0:32, :])
                nc.sync.dma_start(dst_t[64:128, :], dst_t[0:64, :])

            # dinv-scaled identity table: row j*98+g = dinv[g*128+j]*onehot(j)
            for g in range(NT):
                nc.vector.tensor_scalar(
                    tstage[:, g * 128:(g + 1) * 128], id_sb[:],
                    dinvc[:, g:g + 1], None, mult)
            nc.sync.dma_start(table_d[:], tstage[:])

            qrr = [0]

            def next_q():
                qrr[0] = (qrr[0] + 1) % 4
                return qrr[0]

            for l in range(DEPTH):
                yf = y_full[l % 2]
                yl = y_loc[l % 2]
                with tc.nc.named_scope(f"L{l}_y"):
                    for g in range(NT):
                        h = qm.tile([128, F], f32, tag="h")
                        nc.tensor.matmul(h[:], xT[:, g * 128:(g + 1) * 128],
                                         w_sb[:], start=True, stop=True)
                        nc.scalar.activation(
                            ystage[:, g * 128:g * 128 + F], h[:], Copy,
                            scale=dinvc[:, g:g + 1])
                    nc.sync.dma_start(yl[:], ystage[:])
                with tc.nc.named_scope(f"L{l}_ag"):
                    nc.gpsimd.collective_compute(
                        "AllGather", mybir.AluOpType.bypass,
                        replica_groups=[list(range(NC))],
                        ins=[yl[:]], outs=[yf[:]])
                with tc.nc.named_scope(f"L{l}_edge"):
                    for tiles, base, kk in chunks:
                        # 4 agg accumulators share one 2KB PSUM bank tile
                        nbank = (len(tiles) + 3) // 4
                        banks = [qa.tile([64, 512], f32, tag="agg",
                                         name="agg") for _ in range(nbank)]
                        # start=True clears has_written for the whole bank, so
                        # open each bank once with a zeroing matmul and have
                        # every real matmul accumulate (start=False)
                        for bk in banks:
                            nc.tensor.matmul(bk[:], zt[:, 0:64], zt[:],
                                             start=True, stop=False)
                        aggs = {}
                        done = {}
                        total = {}
                        for i, (g, nsl) in enumerate(tiles):
                            aggs[g] = banks[i // 4][:, (i % 4) * 128:
                                                    (i % 4 + 1) * 128]
                            done[g] = 1
                            total[g] = sum(nsl) + 1
                            # self-loop: y_tile^T @ (dinv-scaled identity)
                            nc.tensor.matmul(
                                aggs[g][:],
                                ystage[:, g * 128:g * 128 + F],
                                tstage[:, g * 128:(g + 1) * 128],
                                start=False, stop=(done[g] == total[g]))
                        for ww in range(NWIN):
                            k = kk[ww]
                            if k == 0:
                                continue
                            b0 = base[ww]
                            gmt = ip.tile([128, KMAX // 16], i16, tag="gm")
                            nc.sync.dma_start(
                                gmt[:, :k // 16],
                                gm_rep[:, b0 // 16:(b0 + k) // 16])
                            msg = mp.tile([128, KMAX // 128, 128], bf16,
                                          tag="msg")
                            nc.gpsimd.dma_gather(
                                msg[:, :k // 128, :],
                                yf[ww * WIN:(ww + 1) * WIN, :],
                                gmt[:, :k // 16], k, k, 128,
                                transpose=False, single_packet=False,
                                queue_num=next_q())
                            sel = sp_.tile([128, KMAX // 128, 128], bf16,
                                           tag="sel")
                            cache_v = sel_cache[b0:b0 + k, :].rearrange(
                                "(c p) f -> p c f", p=128)
                            if l == 0:
                                # gather sel one-hots once, stash to DRAM
                                gst = ip.tile([128, KMAX // 16], i16,
                                              tag="gs")
                                nc.sync.dma_start(
                                    gst[:, :k // 16],
                                    gs_rep[:, b0 // 16:(b0 + k) // 16])
                                nc.gpsimd.dma_gather(
                                    sel[:, :k // 128, :], table_d[:],
                                    gst[:, :k // 16], k, k, 128,
                                    transpose=False, single_packet=False,
                                    queue_num=next_q())
                                nc.sync.dma_start(cache_v,
                                                  sel[:, :k // 128, :])
                            else:
                                # layers 1+: plain HWDGE reload (no desc-gen)
                                nc.sync.dma_start(sel[:, :k // 128, :],
                                                  cache_v)
                            off = 0
                            for g, nsl in tiles:
                                for u in range(nsl[ww]):
                                    done[g] += 1
                                    nc.tensor.matmul(
                                        aggs[g][:],
                                        msg[:, off + u, 0:64],
                                        sel[:, off + u, :],
                                        start=False,
                                        stop=(done[g] == total[g]))
                                off += nsl[ww]
                        for g, nsl in tiles:
                            if l < DEPTH - 1:
                                nc.scalar.activation(
                                    xT[:, g * 128:(g + 1) * 128],
                                    aggs[g][:], Relu, bias=b_sb[:])
                            else:
                                fr = op_.tile([64, 128], f32, tag="fr")
                                nc.scalar.activation(fr[:], aggs[g][:],
                                                     Relu, bias=b_sb[:])
                                nc.scalar.dma_start(
                                    out_d[:, g * 128:(g + 1) * 128], fr[:])

    nc.compile()
    return nc


def _host_inputs(x, W, b, edge_index):
    import ml_dtypes
    deg = np.bincount(np.asarray(edge_index[1], np.int64),
                      minlength=N).astype(np.float64) + 1.0
    dinv_full = (1.0 / np.sqrt(deg)).astype(np.float32)
    chunks, S, msg_all, sel_all = _build_schedule(np.asarray(edge_index))

    x = np.asarray(x, np.float32)
    in_maps = []
    for c in range(NC):
        xp = np.zeros((NP, F), np.float32)
        xp[:NLOC] = x[c * NLOC:(c + 1) * NLOC]
        dinv_slot = np.zeros(NP, np.float32)
        dinv_slot[:NLOC] = dinv_full[c * NLOC:(c + 1) * NLOC]
        m = {"xt": np.ascontiguousarray(xp.T).astype(ml_dtypes.bfloat16),
             "W": np.asarray(W, np.float32).astype(ml_dtypes.bfloat16),
             "b": np.asarray(b, np.float32).reshape(64, 1),
             "ident": np.eye(128, dtype=ml_dtypes.bfloat16),
             "dinvc": np.ascontiguousarray(dinv_slot.reshape(NT, 128).T),
             "gm": _wrap16(msg_all[c]),
             "gs": _wrap16(sel_all[c])}
        in_maps.append(m)
    return chunks, S, in_maps


last_results = None
last_exec_ns = None


def kernel(x, edge_index, batch_index, node_rankings, W, b, **_unused):
    import os
    import time
    from concourse.bass_utils import run_bass_kernel_spmd

    global last_results, last_exec_ns
    chunks, S, in_maps = _host_inputs(x, W, b, np.asarray(edge_index))
    nc = _build_program(chunks, S)

    cores = list(range(NC))
    captured = {}
    if os.environ.get("KERNEL_TIME"):
        import jax
        orig_jit = jax.jit

        def spy_jit(*a, **kw):
            f = orig_jit(*a, **kw)

            def wrapper(*args):
                captured["fn"], captured["args"] = f, args
                return f(*args)
            return wrapper
        jax.jit = spy_jit
    try:
        if os.environ.get("KERNEL_TRACE"):
            try:
                res = run_bass_kernel_spmd(nc, in_maps, cores, trace=True)
            except Exception:
                import traceback
                traceback.print_exc()
                res = run_bass_kernel_spmd(nc, in_maps, cores)
        else:
            res = run_bass_kernel_spmd(nc, in_maps, cores)
    finally:
        if captured:
            import jax
            jax.jit = orig_jit
    if captured.get("fn") is not None:
        # warm re-execution of the captured jitted NEFF call: wall time is
        # upload + execute + sync, no retrace/compile
        import jax
        t0 = time.perf_counter()
        o = captured["fn"](*captured["args"])
        jax.block_until_ready(o)
        last_exec_ns = int((time.perf_counter() - t0) * 1e9)
    last_results = res

    out = np.empty((N, F), np.float32)
    for c in range(NC):
        out[c * NLOC:(c + 1) * NLOC] = res.results[c]["out"].T[:NLOC]
    return out
